# revision 4
# baseline (speedup 1.0000x reference)
"""Trainium2 Bass kernel for Erosion2D (tf.nn.erosion2d, stride 1, SAME, NHWC).

  out[b,y,x,c] = min_{dy,dx} xpad[b, y+dy, x+dx, c] - w[3-dy, 3-dx, c]
  x: (8, 512, 512, 32) f32, w: (4,4,32) f32, +inf padding, 4x4 window.

Sharding: pure data parallel - batch element b runs on NeuronCore b (8 cores).

Per-core layout: partition p = band*32 + c (4 H-bands x 32 channels), the
padded (rows, cols) of the band slab in the free dimension - every one of the
16 taps is then just a free-dim offset of one SBUF tile.

v2 schedule (16 chunks of 8 output rows per band), 6 output chains:
  A..D (dy rows): ScalarE activation tap (odd dx, bias=-w) starts the chain,
    then two DVE scalar_tensor_tensor ops fuse (x - w) with the min-accum
    for the even-dx taps (4B-aligned -> 2x bf16 mode).
  E: ScalarE tap (0,3) min DVE tensor_scalar tap (2,3) via GPSIMD tensor_tensor.
  F: GPSIMD tensor_scalar tap (1,3) min DVE tap (3,3) via GPSIMD tensor_tensor.
  6 partial outputs DMA'd out as bf16; host min-reduces them in f32.

Per-chunk engine budget: ScalarE 5 acts (18.5us), DVE 2 ts + 8 stt (21.9us),
GPSIMD 1 ts + 2 tt (21us), DMA 7.7MB (21us) - all four lanes ~balanced.
"""

import numpy as np
import ml_dtypes

import concourse.bacc as bacc
import concourse.mybir as mybir
from concourse.tile import TileContext
from concourse.bass_utils import run_bass_kernel_spmd

BIG = np.float32(1e30)

B, H, W, C = 8, 512, 512, 32
KH, KW = 4, 4
NBAND = 4
BAND_H = H // NBAND              # 128 rows per band
HP = H + KH - 1                  # 515 padded rows
WPAD = 516                       # padded cols, even (covers dx 0..3 + 511)
SLAB_ROWS = BAND_H + KH - 1      # 131 rows per band incl. halo
RB = 8                           # output rows per chunk
NOUT = 6                         # partial outputs (host min-reduces them)

_CACHED_NC = None


def _build_nc():
    global _CACHED_NC
    if _CACHED_NC is not None:
        return _CACHED_NC
    rb = RB
    n_chunks = BAND_H // rb
    slab = rb + KH - 1

    nc = bacc.Bacc("TRN2", target_bir_lowering=False, debug=False, num_devices=8)
    x_d = nc.declare_dram_parameter("x", [128, SLAB_ROWS, WPAD], mybir.dt.bfloat16, isOutput=False)
    w_d = nc.declare_dram_parameter("w", [128, 32], mybir.dt.float32, isOutput=False)
    o_d = [
        nc.declare_dram_parameter(f"o{c}", [128, BAND_H, W], mybir.dt.bfloat16, isOutput=True)
        for c in range(NOUT)
    ]

    amin = mybir.AluOpType.min
    asub = mybir.AluOpType.subtract
    ident = mybir.ActivationFunctionType.Identity

    with TileContext(nc) as tc:
        with (
            tc.tile_pool(name="wpool", bufs=1) as wpool,
            tc.tile_pool(name="evpool", bufs=3) as evpool,
            tc.tile_pool(name="tmp_pool", bufs=2) as tmp_pool,
            tc.tile_pool(name="accpool", bufs=2) as accpool,
        ):
            w_tile = wpool.tile([128, 32], mybir.dt.float32)
            nc.sync.dma_start(out=w_tile[:], in_=w_d[:, :])

            def wneg(dy, dx):   # -w for ScalarE bias (added)
                t = 4 * dy + dx
                return w_tile[:, t : t + 1]

            def wpos(dy, dx):   # +w for subtract ops
                t = 16 + 4 * dy + dx
                return w_tile[:, t : t + 1]

            for k in range(n_chunks):
                r0 = rb * k
                xe = evpool.tile([128, slab, WPAD], mybir.dt.bfloat16, tag="xe")
                nc.sync.dma_start(out=xe[:], in_=x_d[:, r0 : r0 + slab, :])

                def view(dy, dx):
                    return xe[:, dy : dy + rb, dx : dx + W]

                acc = {
                    ch: accpool.tile(
                        [128, rb, W], mybir.dt.bfloat16, tag=f"acc{ch}", name=f"acc{ch}"
                    )
                    for ch in "ABCDEF"
                }
                tmp1 = tmp_pool.tile([128, rb, W], mybir.dt.bfloat16, tag="tmp1")
                tmp2 = tmp_pool.tile([128, rb, W], mybir.dt.bfloat16, tag="tmp2")

                # GPSIMD: three odd-dx taps (E/F chain material), 1-src subs.
                nc.gpsimd.tensor_scalar_sub(tmp1[:], view(1, 3), wpos(1, 3))
                nc.gpsimd.tensor_scalar_sub(acc["F"][:], view(2, 3), wpos(2, 3))
                nc.gpsimd.tensor_scalar_sub(tmp2[:], view(3, 3), wpos(3, 3))

                # ScalarE: chain starts (odd dx taps). E first so DVE can join it.
                nc.scalar.activation(acc["E"][:], view(0, 3), ident, bias=wneg(0, 3))
                for ch, dy in (("A", 0), ("B", 1), ("C", 2), ("D", 3)):
                    nc.scalar.activation(acc[ch][:], view(dy, 1), ident, bias=wneg(dy, 1))

                # DVE: 8 fused (x - w) min acc taps, even dx (2x bf16 stt),
                # then the two E/F min-joins as plain tensor_tensor.
                for dx in (0, 2):
                    for ch, dy in (("A", 0), ("B", 1), ("C", 2), ("D", 3)):
                        nc.vector.scalar_tensor_tensor(
                            acc[ch][:], view(dy, dx), wpos(dy, dx), acc[ch][:],
                            asub, amin,
                        )
                nc.vector.tensor_tensor(acc["E"][:], acc["E"][:], tmp1[:], amin)
                nc.vector.tensor_tensor(acc["F"][:], acc["F"][:], tmp2[:], amin)

                for c, ch in enumerate("ABCDEF"):
                    nc.sync.dma_start(out=o_d[c][:, r0 : r0 + rb, :], in_=acc[ch][:])

    nc.finalize()
    _CACHED_NC = nc
    return nc


def _pack_inputs(x, w):
    # reflected weights per tap t=4*dy+dx, replicated over the 4 bands.
    # cols 0..15: -w (ScalarE bias, added); cols 16..31: +w (subtract ops).
    wtab = np.empty((128, 32), np.float32)
    for dy in range(KH):
        for dx in range(KW):
            t = 4 * dy + dx
            wr = np.tile(w[KH - 1 - dy, KW - 1 - dx, :], NBAND)
            wtab[:, t] = -wr
            wtab[:, 16 + t] = wr

    in_maps = []
    for m in range(B):
        xp = np.full((HP, WPAD, C), BIG, np.float32)
        xp[1 : 1 + H, 1 : 1 + W, :] = x[m]
        bands = np.stack([xp[BAND_H * b : BAND_H * b + SLAB_ROWS] for b in range(NBAND)])
        arr = np.ascontiguousarray(bands.transpose(0, 3, 1, 2)).reshape(128, SLAB_ROWS, WPAD)
        in_maps.append({"x": arr.astype(ml_dtypes.bfloat16), "w": wtab})
    return in_maps


def _unpack_outputs(results):
    out = np.empty((B, H, W, C), np.float32)
    for m in range(B):
        acc = results[m]["o0"].astype(np.float32)
        for c in range(1, NOUT):
            acc = np.minimum(acc, results[m][f"o{c}"].astype(np.float32))
        out[m] = acc.reshape(NBAND, C, BAND_H, W).transpose(0, 2, 3, 1).reshape(H, W, C)
    return out


def kernel(x: np.ndarray, w: np.ndarray) -> np.ndarray:
    x = np.ascontiguousarray(np.asarray(x, dtype=np.float32))
    w = np.ascontiguousarray(np.asarray(w, dtype=np.float32))
    nc = _build_nc()
    in_maps = _pack_inputs(x, w)
    res = run_bass_kernel_spmd(nc, in_maps, core_ids=list(range(8)))
    return _unpack_outputs(res.results)


# revision 5
# speedup vs baseline: 6.9994x; 6.9994x over previous
"""Trainium2 Bass kernel for Erosion2D (tf.nn.erosion2d, stride 1, SAME, NHWC).

  out[b,y,x,c] = min_{dy,dx} xpad[b, y+dy, x+dx, c] - w[3-dy, 3-dx, c]
  x: (8, 512, 512, 32) f32, w: (4,4,32) f32, +inf padding, 4x4 window.

Sharding: pure data parallel - batch element b runs on NeuronCore b (8 cores).

Per-core layout: partition p = band*32 + c (4 H-bands x 32 channels), the
padded (rows, cols) of the band slab in the free dimension - every one of the
16 taps is then just a free-dim offset of one SBUF tile.

v5 schedule: 8 slabs of 16 output rows (19 incl. halo), ops at FD=8192 to
amortize per-instruction overhead (ScalarE 352cyc, DVE 58cyc + DRAIN):
  chains A..G: ScalarE activation odd-dx tap (bias=-w) starts the chain;
    DVE tensor_scalar even-dx tap (4x bf16) + tensor_tensor min join (2x).
  chain H: two leftover taps: DVE ts (3,3) at 2x_2P + ts (3,2) at 4x + tt.
  8 partial outputs DMA'd out as bf16; host min-reduces them in f32.
acc tiles are single-buffered (SBUF limit): chain X's act for slab p+1
waits on X's out-DMA of slab p, which fires ~40us earlier - no stall.

Per-slab budget: ScalarE 7 acts (49.8us), DVE 10 ts/tt (56.5us),
DMA 19.9MB in + 134MB out (51.5us/slab) - DVE-bound ~452us projected.
"""

import numpy as np
import ml_dtypes

import concourse.bacc as bacc
import concourse.mybir as mybir
from concourse.tile import TileContext
from concourse.bass_utils import run_bass_kernel_spmd

BIG = np.float32(1e30)

B, H, W, C = 8, 512, 512, 32
KH, KW = 4, 4
NBAND = 4
BAND_H = H // NBAND              # 128 rows per band
HP = H + KH - 1                  # 515 padded rows
WPAD = 516                       # padded cols, even (covers dx 0..3 + 511)
SLAB_ROWS = BAND_H + KH - 1      # 131 rows per band incl. halo
RB = 16                          # output rows per slab
NOUT = 8                         # partial outputs (host min-reduces them)

# chains A..G: (odd-dx tap for ScalarE start, even-dx tap for DVE ts+tt)
CHAINS = [
    ((0, 1), (0, 0)),
    ((1, 1), (1, 0)),
    ((2, 1), (2, 0)),
    ((3, 1), (3, 0)),
    ((0, 3), (0, 2)),
    ((1, 3), (1, 2)),
    ((2, 3), (2, 2)),
]
# chain H: both taps on DVE ((3,3) at 2x_2P, (3,2) at 4x)
H_TAPS = ((3, 3), (3, 2))

_CACHED_NC = None


def _build_nc():
    global _CACHED_NC
    if _CACHED_NC is not None:
        return _CACHED_NC
    rb = RB
    n_slabs = BAND_H // rb
    slab = rb + KH - 1

    nc = bacc.Bacc("TRN2", target_bir_lowering=False, debug=False, num_devices=8)
    x_d = nc.declare_dram_parameter("x", [128, SLAB_ROWS, WPAD], mybir.dt.bfloat16, isOutput=False)
    w_d = nc.declare_dram_parameter("w", [128, 32], mybir.dt.float32, isOutput=False)
    o_d = [
        nc.declare_dram_parameter(f"o{c}", [128, BAND_H, W], mybir.dt.bfloat16, isOutput=True)
        for c in range(NOUT)
    ]

    amin = mybir.AluOpType.min
    ident = mybir.ActivationFunctionType.Identity

    with TileContext(nc) as tc:
        with (
            tc.tile_pool(name="wpool", bufs=1) as wpool,
            tc.tile_pool(name="evpool", bufs=2) as evpool,
            tc.tile_pool(name="tmp_pool", bufs=2) as tmp_pool,
            tc.tile_pool(name="accpool", bufs=1) as accpool,
        ):
            w_tile = wpool.tile([128, 32], mybir.dt.float32)
            nc.sync.dma_start(out=w_tile[:], in_=w_d[:, :])

            def wneg(dy, dx):   # -w for ScalarE bias (added)
                t = 4 * dy + dx
                return w_tile[:, t : t + 1]

            def wpos(dy, dx):   # +w for tensor_scalar_sub
                t = 16 + 4 * dy + dx
                return w_tile[:, t : t + 1]

            for k in range(n_slabs):
                r0 = rb * k
                xe = evpool.tile([128, slab, WPAD], mybir.dt.bfloat16, tag="xe")
                nc.sync.dma_start(out=xe[:], in_=x_d[:, r0 : r0 + slab, :])

                def view(dy, dx):
                    return xe[:, dy : dy + rb, dx : dx + W]

                acc = [
                    accpool.tile(
                        [128, rb, W], mybir.dt.bfloat16, tag=f"acc{c}", name=f"acc{c}"
                    )
                    for c in range(NOUT)
                ]

                # chain H first on DVE so its out-DMA frees early.
                (dya, dxa), (dyb, dxb) = H_TAPS
                nc.vector.tensor_scalar_sub(acc[7][:], view(dya, dxa), wpos(dya, dxa))
                tmp = tmp_pool.tile([128, rb, W], mybir.dt.bfloat16, tag="tmp", name="tmpH")
                nc.vector.tensor_scalar_sub(tmp[:], view(dyb, dxb), wpos(dyb, dxb))
                nc.vector.tensor_tensor(acc[7][:], acc[7][:], tmp[:], amin)
                nc.sync.dma_start(out=o_d[7][:, r0 : r0 + rb, :], in_=acc[7][:])

                for c, (ta, td) in enumerate(CHAINS):
                    dy, dx = ta
                    nc.scalar.activation(
                        acc[c][:], view(dy, dx), ident, bias=wneg(dy, dx)
                    )
                    tmp = tmp_pool.tile(
                        [128, rb, W], mybir.dt.bfloat16, tag="tmp", name="tmp"
                    )
                    dy, dx = td
                    nc.vector.tensor_scalar_sub(tmp[:], view(dy, dx), wpos(dy, dx))
                    nc.vector.tensor_tensor(acc[c][:], acc[c][:], tmp[:], amin)
                    nc.sync.dma_start(out=o_d[c][:, r0 : r0 + rb, :], in_=acc[c][:])

    nc.finalize()
    _CACHED_NC = nc
    return nc


def _pack_inputs(x, w):
    # reflected weights per tap t=4*dy+dx, replicated over the 4 bands.
    # cols 0..15: -w (ScalarE bias, added); cols 16..31: +w (ts_sub).
    wtab = np.empty((128, 32), np.float32)
    for dy in range(KH):
        for dx in range(KW):
            t = 4 * dy + dx
            wr = np.tile(w[KH - 1 - dy, KW - 1 - dx, :], NBAND)
            wtab[:, t] = -wr
            wtab[:, 16 + t] = wr

    in_maps = []
    for m in range(B):
        xp = np.full((HP, WPAD, C), BIG, np.float32)
        xp[1 : 1 + H, 1 : 1 + W, :] = x[m]
        bands = np.stack([xp[BAND_H * b : BAND_H * b + SLAB_ROWS] for b in range(NBAND)])
        arr = np.ascontiguousarray(bands.transpose(0, 3, 1, 2)).reshape(128, SLAB_ROWS, WPAD)
        in_maps.append({"x": arr.astype(ml_dtypes.bfloat16), "w": wtab})
    return in_maps


def _unpack_outputs(results):
    out = np.empty((B, H, W, C), np.float32)
    for m in range(B):
        acc = results[m]["o0"].astype(np.float32)
        for c in range(1, NOUT):
            acc = np.minimum(acc, results[m][f"o{c}"].astype(np.float32))
        out[m] = acc.reshape(NBAND, C, BAND_H, W).transpose(0, 2, 3, 1).reshape(H, W, C)
    return out


def kernel(x: np.ndarray, w: np.ndarray) -> np.ndarray:
    x = np.ascontiguousarray(np.asarray(x, dtype=np.float32))
    w = np.ascontiguousarray(np.asarray(w, dtype=np.float32))
    nc = _build_nc()
    in_maps = _pack_inputs(x, w)
    res = run_bass_kernel_spmd(nc, in_maps, core_ids=list(range(8)))
    return _unpack_outputs(res.results)
